# revision 10
# baseline (speedup 1.0000x reference)
"""Masked-MVN (eye covariance) NLL loss on 8 Trainium2 cores.

loss = 0.5 * ( sum(eps^2 * (y != 0)) / (s * B) + D * (log(2*pi) + log(s)) )
with s = softplus(sigma), B = 256, D = 24*4096.

The heavy part (201 MB masked sum-of-squares) runs data-parallel on 8
NeuronCores; the O(1) scalar epilogue runs on host. eps/y chunks are
packed interleaved into one input tensor per core so each chunk lands in
a single DMA (the fused DVE op only supports a small number of sync
waits, and bigger DMAs are faster anyway).
"""

import sys

for _p in ("/opt/trn_rl_repo",):
    if _p not in sys.path:
        sys.path.insert(0, _p)

import numpy as np

B, Q, N = 256, 24, 4096
NCORES = 8
BSH = B // NCORES            # 32 batches per core
P = 128                      # SBUF partitions
M = BSH * Q * N // P         # 24576 floats per partition per tensor
CHUNK = 2048                 # free-dim chunk per tensor per tile
NCHUNK = M // CHUNK          # 12
NBUF = 8                     # == number of HW DMA queues (same-queue WAW elision)
D = Q * N                    # 98304 (MVN event dim)

_CACHE = {}


def _build_nc():
    import concourse.bass as bass
    import concourse.mybir as mybir
    import concourse.tile as tile

    nc = bass.Bass()
    xy = nc.dram_tensor("xy", [P, 2 * M], mybir.dt.float32, kind="ExternalInput")
    out = nc.dram_tensor("out", [P, NCHUNK], mybir.dt.float32, kind="ExternalOutput")

    with tile.TileContext(nc) as tc:
        with (
            tc.tile_pool(name="io", bufs=4) as io_pool,
            tc.tile_pool(name="me", bufs=NCHUNK) as me_pool,
            tc.tile_pool(name="sq", bufs=2) as sq_pool,
            tc.tile_pool(name="acc", bufs=1) as acc_pool,
        ):
            part = acc_pool.tile([P, NCHUNK], mybir.dt.float32)
            for j in range(NCHUNK):
                xyt = io_pool.tile([P, 2 * CHUNK], mybir.dt.float32, tag="xy")
                nc.sync.dma_start(xyt[:], xy[:, bass.ts(j, 2 * CHUNK)])
                e = xyt[:, 0:CHUNK]
                yt = xyt[:, CHUNK : 2 * CHUNK]

                # me = (y != 0) * eps  — one DVE pass
                me = me_pool.tile([P, CHUNK], mybir.dt.float32, tag="me")
                nc.vector.scalar_tensor_tensor(
                    me[:],
                    yt,
                    0.0,
                    e,
                    op0=mybir.AluOpType.not_equal,
                    op1=mybir.AluOpType.mult,
                )
                # part[:, j] = sum(me^2) — one ACT pass (fused square+reduce)
                sq = sq_pool.tile([P, CHUNK], mybir.dt.float32, tag="sq")
                nc.scalar.activation(
                    sq[:],
                    me[:],
                    mybir.ActivationFunctionType.Square,
                    accum_out=part[:, j : j + 1],
                )
            nc.sync.dma_start(out[:], part[:])

    _split_waits(nc, mybir)
    return nc


def _split_waits(nc, mybir):
    """Walrus codegen in this container only accepts ONE sync wait per
    engine/DMA instruction. Hoist extra waits onto InstNoOp instructions
    inserted just before, on the same engine stream (engines execute
    in order, so wait-on-nop then wait-on-inst is equivalent)."""
    f = nc.m.functions[0]
    for blk in f.blocks:
        fixes = []
        for idx, inst in enumerate(blk.instructions):
            si = getattr(inst, "sync_info", None)
            if si is None or not si.on_wait or len(si.on_wait) <= 1:
                continue
            fixes.append((idx, inst))
        if not fixes:
            continue
        result = list(blk.instructions)
        for idx, inst in reversed(fixes):
            waits = list(inst.sync_info.on_wait)
            nops = []
            for w in waits[:-1]:
                bi = nc.engines[inst.engine].nop(hint="wait-hoist")
                nop_inst = bi.ins
                for b2 in f.blocks:
                    if nop_inst in b2.instructions:
                        b2.instructions.remove(nop_inst)
                        break
                else:
                    raise AssertionError("hoist nop not found in any block")
                nop_inst.sync_info = mybir.SyncInfo(on_wait=[w], on_update=[])
                nops.append(nop_inst)
            inst.sync_info = mybir.SyncInfo(
                on_wait=[waits[-1]], on_update=list(inst.sync_info.on_update)
            )
            result[idx:idx] = nops
        blk.instructions = result


def _pack(eps_t, y_t):
    """[NCORES, P, 2*M] with per-chunk interleave: [eps_j | y_j] blocks."""
    e = np.ascontiguousarray(eps_t, dtype=np.float32).reshape(NCORES, P, NCHUNK, CHUNK)
    y = np.ascontiguousarray(y_t, dtype=np.float32).reshape(NCORES, P, NCHUNK, CHUNK)
    xy = np.empty((NCORES, P, NCHUNK, 2, CHUNK), dtype=np.float32)
    xy[:, :, :, 0, :] = e
    xy[:, :, :, 1, :] = y
    return xy.reshape(NCORES, P, 2 * M)


def _execute(in_maps, trace=False):
    from concourse.bass_utils import run_bass_kernel_spmd

    if "nc" not in _CACHE:
        _CACHE["nc"] = _build_nc()
    nc = _CACHE["nc"]
    return run_bass_kernel_spmd(nc, in_maps, core_ids=list(range(NCORES)), trace=trace)


def kernel(eps_t, y_t, sigma):
    xy = _pack(eps_t, y_t)
    in_maps = [{"xy": xy[i]} for i in range(NCORES)]
    res = _execute(in_maps)
    total = float(sum(np.asarray(r["out"], dtype=np.float64).sum() for r in res.results))

    sig = float(np.asarray(sigma, dtype=np.float64).reshape(-1)[0])
    # softplus(sigma), numerically stable
    s = np.logaddexp(0.0, sig)
    loss = 0.5 * (total / (s * B) + D * (np.log(2.0 * np.pi) + np.log(s)))
    return np.asarray(loss, dtype=np.float32)


# revision 11
# speedup vs baseline: 1.1807x; 1.1807x over previous
"""Masked-MVN (eye covariance) NLL loss on 8 Trainium2 cores.

loss = 0.5 * ( sum(eps^2 * (y != 0)) / (s * B) + D * (log(2*pi) + log(s)) )
with s = softplus(sigma), B = 256, D = 24*4096.

The heavy part (201 MB masked sum-of-squares) runs data-parallel on 8
NeuronCores; the O(1) scalar epilogue runs on host. eps/y chunks are
packed interleaved into one input tensor per core so each chunk lands in
a single DMA (the fused DVE op only supports a small number of sync
waits, and bigger DMAs are faster anyway).
"""

import sys

for _p in ("/opt/trn_rl_repo",):
    if _p not in sys.path:
        sys.path.insert(0, _p)

import numpy as np

B, Q, N = 256, 24, 4096
NCORES = 8
BSH = B // NCORES            # 32 batches per core
P = 128                      # SBUF partitions
M = BSH * Q * N // P         # 24576 floats per partition per tensor
CHUNK = 2048                 # free-dim chunk per tensor per tile
NCHUNK = M // CHUNK          # 12
NBUF = 8                     # == number of HW DMA queues (same-queue WAW elision)
D = Q * N                    # 98304 (MVN event dim)

_CACHE = {}


def _build_nc():
    import concourse.bass as bass
    import concourse.mybir as mybir
    import concourse.tile as tile

    nc = bass.Bass()
    xy = nc.dram_tensor("xy", [P, 2 * M], mybir.dt.float32, kind="ExternalInput")
    out = nc.dram_tensor("out", [P, NCHUNK], mybir.dt.float32, kind="ExternalOutput")

    with tile.TileContext(nc) as tc:
        with (
            tc.tile_pool(name="io", bufs=NBUF) as io_pool,
            tc.tile_pool(name="sq", bufs=2) as sq_pool,
            tc.tile_pool(name="acc", bufs=1) as acc_pool,
        ):
            part = acc_pool.tile([P, NCHUNK], mybir.dt.float32)
            for j in range(NCHUNK):
                xyt = io_pool.tile([P, 2 * CHUNK], mybir.dt.float32, tag="xy")
                nc.sync.dma_start(xyt[:], xy[:, bass.ts(j, 2 * CHUNK)])
                e = xyt[:, 0:CHUNK]
                yt = xyt[:, CHUNK : 2 * CHUNK]

                # e <- (y != 0) * eps  — one DVE pass, in place over the eps half
                nc.vector.scalar_tensor_tensor(
                    e,
                    yt,
                    0.0,
                    e,
                    op0=mybir.AluOpType.not_equal,
                    op1=mybir.AluOpType.mult,
                )
                # part[:, j] = sum(e^2) — one ACT pass (fused square+reduce)
                sq = sq_pool.tile([P, CHUNK], mybir.dt.float32, tag="sq")
                nc.scalar.activation(
                    sq[:],
                    e,
                    mybir.ActivationFunctionType.Square,
                    accum_out=part[:, j : j + 1],
                )
            nc.sync.dma_start(out[:], part[:])

    _split_waits(nc, mybir)
    return nc


def _split_waits(nc, mybir):
    """Walrus codegen in this container only accepts ONE sync wait per
    engine/DMA instruction. Hoist extra waits onto InstNoOp instructions
    inserted just before, on the same engine stream (engines execute
    in order, so wait-on-nop then wait-on-inst is equivalent)."""
    f = nc.m.functions[0]
    for blk in f.blocks:
        fixes = []
        for idx, inst in enumerate(blk.instructions):
            si = getattr(inst, "sync_info", None)
            if si is None or not si.on_wait or len(si.on_wait) <= 1:
                continue
            fixes.append((idx, inst))
        if not fixes:
            continue
        result = list(blk.instructions)
        for idx, inst in reversed(fixes):
            waits = list(inst.sync_info.on_wait)
            nops = []
            for w in waits[:-1]:
                bi = nc.engines[inst.engine].nop(hint="wait-hoist")
                nop_inst = bi.ins
                for b2 in f.blocks:
                    if nop_inst in b2.instructions:
                        b2.instructions.remove(nop_inst)
                        break
                else:
                    raise AssertionError("hoist nop not found in any block")
                nop_inst.sync_info = mybir.SyncInfo(on_wait=[w], on_update=[])
                nops.append(nop_inst)
            inst.sync_info = mybir.SyncInfo(
                on_wait=[waits[-1]], on_update=list(inst.sync_info.on_update)
            )
            result[idx:idx] = nops
        blk.instructions = result


def _pack(eps_t, y_t):
    """[NCORES, P, 2*M] with per-chunk interleave: [eps_j | y_j] blocks."""
    e = np.ascontiguousarray(eps_t, dtype=np.float32).reshape(NCORES, P, NCHUNK, CHUNK)
    y = np.ascontiguousarray(y_t, dtype=np.float32).reshape(NCORES, P, NCHUNK, CHUNK)
    xy = np.empty((NCORES, P, NCHUNK, 2, CHUNK), dtype=np.float32)
    xy[:, :, :, 0, :] = e
    xy[:, :, :, 1, :] = y
    return xy.reshape(NCORES, P, 2 * M)


def _execute(in_maps, trace=False):
    from concourse.bass_utils import run_bass_kernel_spmd

    if "nc" not in _CACHE:
        _CACHE["nc"] = _build_nc()
    nc = _CACHE["nc"]
    return run_bass_kernel_spmd(nc, in_maps, core_ids=list(range(NCORES)), trace=trace)


def kernel(eps_t, y_t, sigma):
    xy = _pack(eps_t, y_t)
    in_maps = [{"xy": xy[i]} for i in range(NCORES)]
    res = _execute(in_maps)
    total = float(sum(np.asarray(r["out"], dtype=np.float64).sum() for r in res.results))

    sig = float(np.asarray(sigma, dtype=np.float64).reshape(-1)[0])
    # softplus(sigma), numerically stable
    s = np.logaddexp(0.0, sig)
    loss = 0.5 * (total / (s * B) + D * (np.log(2.0 * np.pi) + np.log(s)))
    return np.asarray(loss, dtype=np.float32)
